# revision 1
# baseline (speedup 1.0000x reference)
import numpy as np
import jax
import jax.numpy as jnp
from jax import lax

EPS = 1e-5

# Problem shapes (hardcoded per spec): B=8, C=256, H=W=128, J=19, S=512, NH=128.
# Sharding: data-parallel over batch, B=8 -> one image per NeuronCore,
# all parameters replicated (every op is batch-independent).

_BATCH_ARGS = ("x", "seg_labels", "style_codes", "noise")
_ARG_ORDER = (
    "x", "seg_labels", "style_codes", "noise", "noise_var",
    "blending_gamma", "blending_beta", "fc_w", "fc_b",
    "conv_gamma_w", "conv_gamma_b", "conv_beta_w", "conv_beta_b",
    "sp_shared_w", "sp_shared_b", "sp_gamma_w", "sp_gamma_b",
    "sp_beta_w", "sp_beta_b",
)


def _conv(x, w, b):
    y = lax.conv_general_dilated(
        x, w, window_strides=(1, 1), padding="SAME",
        dimension_numbers=("NCHW", "OIHW", "NCHW"),
    )
    return y + b[None, :, None, None]


def _instance_norm(x):
    m = jnp.mean(x, axis=(2, 3), keepdims=True)
    v = jnp.var(x, axis=(2, 3), keepdims=True)
    return (x - m) * lax.rsqrt(v + EPS)


def _forward(x, seg_labels, style_codes, noise, noise_var, blending_gamma,
             blending_beta, fc_w, fc_b, conv_gamma_w, conv_gamma_b,
             conv_beta_w, conv_beta_b, sp_shared_w, sp_shared_b,
             sp_gamma_w, sp_gamma_b, sp_beta_w, sp_beta_b):
    J = fc_w.shape[0]
    added_noise = jnp.transpose(noise * noise_var, (0, 3, 2, 1))
    normalized = _instance_norm(x + added_noise)
    segmap = jnp.transpose(
        jax.nn.one_hot(seg_labels, J, dtype=x.dtype), (0, 3, 1, 2)
    )  # [B,J,H,W]
    mu = jax.nn.relu(jnp.einsum("bji,joi->bjo", style_codes, fc_w) + fc_b)  # [B,J,S]

    # conv(one-hot scatter) == cheap Cin=J conv with per-image collapsed
    # kernels: A[b,j,o,ky,kx] = sum_s conv_w[o,s,ky,kx] * mu[b,j,s].
    a_gamma = jnp.einsum("osyx,bjs->bjoyx", conv_gamma_w, mu)
    a_beta = jnp.einsum("osyx,bjs->bjoyx", conv_beta_w, mu)

    def seg_conv(seg1, ker):  # seg1 [J,H,W], ker [J,O,3,3]
        return lax.conv_general_dilated(
            seg1[None], jnp.transpose(ker, (1, 0, 2, 3)),
            window_strides=(1, 1), padding="SAME",
            dimension_numbers=("NCHW", "OIHW", "NCHW"),
        )[0]

    gamma_avg = (
        jax.vmap(seg_conv)(segmap, a_gamma) + conv_gamma_b[None, :, None, None]
    )
    beta_avg = (
        jax.vmap(seg_conv)(segmap, a_beta) + conv_beta_b[None, :, None, None]
    )

    actv = jax.nn.relu(_conv(segmap, sp_shared_w, sp_shared_b))
    gamma_spade = _conv(actv, sp_gamma_w, sp_gamma_b)
    beta_spade = _conv(actv, sp_beta_w, sp_beta_b)
    ga = jax.nn.sigmoid(blending_gamma)[0]
    ba = jax.nn.sigmoid(blending_beta)[0]
    gamma_final = ga * gamma_avg + (1.0 - ga) * gamma_spade
    beta_final = ba * beta_avg + (1.0 - ba) * beta_spade
    return normalized * (1.0 + gamma_final) + beta_final


def _run_pmap(inputs, devices):
    in_axes = tuple(0 if k in _BATCH_ARGS else None for k in _ARG_ORDER)
    f = jax.pmap(
        lambda *a: _forward(*[
            v[None] if k in _BATCH_ARGS else v
            for k, v in zip(_ARG_ORDER, a)
        ])[0],
        in_axes=in_axes,
        devices=devices,
    )
    args = [inputs[k] for k in _ARG_ORDER]
    out = f(*args)
    return np.asarray(jax.device_get(out)).astype(np.float32)


def kernel(**inputs) -> np.ndarray:
    inputs = {k: np.asarray(v) for k, v in inputs.items()}
    # Shard batch across the 8 NeuronCores, params replicated.
    try:
        devices = jax.devices()[:8]
        if len(devices) == 8:
            return _run_pmap(inputs, devices)
    except Exception:
        pass
    # Fallback: single-device / CPU execution (still correct).
    try:
        out = _forward(*[jnp.asarray(inputs[k]) for k in _ARG_ORDER])
        return np.asarray(jax.device_get(out)).astype(np.float32)
    except Exception:
        with jax.default_device(jax.devices("cpu")[0]):
            out = _forward(*[jnp.asarray(inputs[k]) for k in _ARG_ORDER])
            return np.asarray(jax.device_get(out)).astype(np.float32)
